# revision 10
# baseline (speedup 1.0000x reference)
"""Trainium2 Bass kernel for nn_DifferentiableProjector (volume rendering).

Math (per ray i, samples s=0..S-1, channels c):
    T_excl[s] = exp(-DT * sum_{s'<s} rho[s'])
    T_incl[s] = exp(-DT * sum_{s'<=s} rho[s'])
    w[s]      = T_excl[s] - T_incl[s]        (= T_excl * alpha)
    out[i,c]  = sum_s w[s] * f[i,s,c]

Sharding: data-parallel over rays, 65536 rays -> 8 cores x 8192 rays.

Design (all compute in "transposed space", s on partitions):
  - host casts rho/f to fp16 AND pre-tiles them so each 512-ray tile is a
    fully contiguous [S, C, T] DRAM slab (16 KB/partition rows) -> plain
    max-bandwidth DMAs, alternating the two HWDGE queues (sync/scalar)
  - cumsum over s (the partition axis) via triangular-ones matmuls on
    TensorE (fp32 PSUM); w = exp(-DT*excl) - exp(-DT*incl) with fp32 exps
    (fp16 exps would cancel catastrophically), cast to fp16
  - the big multiply is ONE DVE instruction per tile at 2x (fp16, w
    broadcast on the middle dim so the innermost stride stays 1);
    splitting it only pays the ~0.4us fixed DVE instruction cost 4x
  - segment-reduce over s: 16 accumulating one-hot matmuls on TensorE
    route channel-c column sums into PSUM row c -> [16, T] per tile;
    E_c weight loads are shared across tile pairs (also keeps the PE
    streaming dense, which matters for its DVFS p-state ramp)
  - ScalarE evacuates PSUM -> SBUF in fp16; outputs stream out per
    tile-pair (half the bytes of fp32; host upcasts after the gather)
  - rho preload is split so tile 0's cumsum isn't gated on the full
    2 MiB rho slab

v1 naive DVE: 415 us; v2 (4-way split mul): 116-129 us. This version:
DMA-bound at ~34 MiB/core over ~360 GB/s -> ~98 us floor + ramp/tail.
"""

import numpy as np

import concourse.bass as bass
import concourse.tile as tile
from concourse.bacc import Bacc
from concourse import mybir
from concourse.bass_utils import run_bass_kernel_spmd

H, W, S, C = 256, 256, 128, 16
N = H * W
NCORES = 8
NS = N // NCORES          # rays per core
P = 128                   # partitions (= S)
T = 512                   # rays per tile
DT = (6.0 - 2.0) / S

_cached = {}

# test-harness hooks (ignored by grading path)
TRACE = False
LAST_RESULTS = None

F16 = mybir.dt.float16
F32 = mybir.dt.float32


def _build_nc(ns: int = NS) -> bass.Bass:
    ntiles = ns // T
    nc = Bacc()
    # host supplies tensors pre-transposed AND pre-tiled:
    #   rho [ntiles*S, T]  (tile t rows t*S:(t+1)*S = rho[s, i] slab)
    #   f   [ntiles*S, C*T] (tile t rows = f[s, (c, i)] slab, contiguous)
    rho_d = nc.dram_tensor("rho", [ntiles * S, T], F16, kind="ExternalInput")
    f_d = nc.dram_tensor("f", [ntiles * S, C * T], F16, kind="ExternalInput")
    cst_d = nc.dram_tensor("consts", [P, 2 * P + C * C], F16, kind="ExternalInput")
    out_d = nc.dram_tensor("out", [C, ns], F16, kind="ExternalOutput")

    with tile.TileContext(nc) as tc:
        with (
            tc.tile_pool(name="cpool", bufs=1) as cpool,
            tc.tile_pool(name="fpool", bufs=6) as fpool,
            tc.tile_pool(name="tpool", bufs=3) as tpool,
            tc.tile_pool(name="spool", bufs=4) as spool,
            tc.tile_pool(name="opool", bufs=3) as opool,
            tc.tile_pool(name="psc", bufs=2, space="PSUM") as psc,
            tc.tile_pool(name="pso", bufs=2, space="PSUM") as pso,
        ):
            # consts + rho on GpSimd's queue (cheap dispatches, engine idle)
            consts = cpool.tile([P, 2 * P + C * C], F16)
            nc.gpsimd.dma_start(out=consts, in_=cst_d[:, :])
            u_excl = consts[:, 0:P]
            u_incl = consts[:, P : 2 * P]
            # E_c = consts[:, 2P + 16c : 2P + 16c + 16]: column m one-hot at c
            e_base = 2 * P

            # rho slabs: [s, (tile, i)] fp16, split so the first pair of
            # tiles is ready ~8x sooner than the full slab
            rho_all = cpool.tile([P, ntiles, T], F16)
            rho_r = rho_d[:, :].rearrange("(t s) i -> s t i", s=S)
            nc.gpsimd.dma_start(out=rho_all[:, 0:2, :], in_=rho_r[:, 0:2, :])
            nc.gpsimd.dma_start(
                out=rho_all[:, 2:ntiles, :], in_=rho_r[:, 2:ntiles, :]
            )

            def tile_front(t, nsplit=1):
                """DMA load + w pipeline + big multiply for tile t. nsplit>1
                chunks the multiply so the reduce can start early (used for
                the last tiles, where the post-DMA tail is exposed).

                The f dispatch alternates sync/scalar rings and is issued
                inline (v2 pattern): engine instruction queues run ahead of
                compute, which keeps the HBM stream fed; explicit upfront
                pre-allocation + prefetch dispatch (v3-v5) made the tile
                framework serialize the dispatch streams and starved DMA."""
                fT = fpool.tile([P, C, T], F16, tag="fT")
                f_eng = nc.sync if t % 2 == 0 else nc.scalar
                f_slab = f_d[t * S : (t + 1) * S, :].rearrange(
                    "s (c i) -> s c i", c=C
                )
                f_eng.dma_start(out=fT, in_=f_slab)
                rhoT = rho_all[:, t, :]

                # cumsum over s (partition axis) via triangular matmuls
                pexc = psc.tile([P, T], F32, tag="pexc")
                pinc = psc.tile([P, T], F32, tag="pinc")
                nc.tensor.matmul(pexc, u_excl, rhoT, start=True, stop=True)
                nc.tensor.matmul(pinc, u_incl, rhoT, start=True, stop=True)

                # exps in fp32 (w = e1 - e2 cancels; fp16 here costs ~4% on w)
                e1 = spool.tile([P, T], F32, tag="e1")
                e2 = spool.tile([P, T], F32, tag="e2")
                nc.scalar.activation(
                    e1, pexc, mybir.ActivationFunctionType.Exp, scale=-DT
                )
                nc.scalar.activation(
                    e2, pinc, mybir.ActivationFunctionType.Exp, scale=-DT
                )
                # the sub stays on DVE: measured Pool tensor_tensor is ~4.3us
                # for this shape (4x the cost model), far too slow for the
                # w -> multiply critical path
                w = spool.tile([P, T], F16, tag="w")
                nc.vector.tensor_sub(w, e1, e2)

                # tmp[s, c, i] = fT[s, c, i] * w[s, i] on DVE at 2x
                tmp = tpool.tile([P, C, T], F16, tag="tmp")
                cs = C // nsplit
                for q in range(nsplit):
                    nc.vector.tensor_mul(
                        tmp[:, q * cs : (q + 1) * cs, :],
                        fT[:, q * cs : (q + 1) * cs, :],
                        w[:, None, :].broadcast_to((P, cs, T)),
                    )
                return tmp

            def tile_back(t, tmp_list):
                """Reduce + evacuate for 1-2 tiles, sharing each E_c weight
                load across the tiles' matmuls."""
                psums = [pso.tile([C, T], F32, tag=f"po{i}", name=f"po{i}_{t}")
                         for i in range(len(tmp_list))]
                for c in range(C):
                    lhs = consts[:, e_base + c * C : e_base + (c + 1) * C]
                    for tmp_t, po in zip(tmp_list, psums):
                        nc.tensor.matmul(
                            po,
                            lhs,
                            tmp_t[:, c, :],
                            start=(c == 0),
                            stop=(c == C - 1),
                        )
                nt = len(tmp_list)
                out_pair = opool.tile([C, nt * T], F16, tag="out_pair",
                                      name=f"out_pair_{t}")
                for k, po in enumerate(psums):
                    nc.scalar.activation(
                        out_pair[:, k * T : (k + 1) * T],
                        po,
                        mybir.ActivationFunctionType.Copy,
                    )
                # stream the output out now (overlaps later tiles); Scalar
                # dispatch right after its own evacs adds no new waits
                nc.scalar.dma_start(
                    out=out_d[:, t * T : (t + nt) * T],
                    in_=out_pair,
                )

            for t in range(0, ntiles - 2, 2):
                tmp_a = tile_front(t)
                tmp_b = tile_front(t + 1)
                tile_back(t, [tmp_a, tmp_b])
            # last two tiles: unpaired, chunked multiplies -> the reduce
            # interleaves with the multiply instead of trailing the last DMA
            tmp_a = tile_front(ntiles - 2, nsplit=4)
            tile_back(ntiles - 2, [tmp_a])
            tmp_b = tile_front(ntiles - 1, nsplit=4)
            tile_back(ntiles - 1, [tmp_b])
    if not nc.is_finalized():
        nc.finalize()
    return nc


def _consts() -> np.ndarray:
    u_excl = np.triu(np.ones((P, P), np.float16), 1)
    u_incl = np.triu(np.ones((P, P), np.float16), 0)
    # E[:, c*C + m] = 1 if m == c else 0  (all rows identical)
    e = np.tile(np.eye(C, dtype=np.float16).reshape(1, C * C), (P, 1))
    return np.ascontiguousarray(np.concatenate([u_excl, u_incl, e], axis=1))


def kernel(rho: np.ndarray, f: np.ndarray) -> np.ndarray:
    global LAST_RESULTS
    if "nc" not in _cached:
        _cached["nc"] = _build_nc()
        _cached["consts"] = _consts()
    nc = _cached["nc"]

    rho16 = np.asarray(rho, dtype=np.float16).reshape(N, S)
    f16 = np.asarray(f, dtype=np.float16).reshape(N, S, C)
    cst = _cached["consts"]
    ntiles = NS // T

    in_maps = []
    for i in range(NCORES):
        sl = slice(i * NS, (i + 1) * NS)
        # [rays, S] -> [ntiles, S, T] / [rays, S, C] -> [ntiles, S, C, T]
        rho_t = np.ascontiguousarray(
            rho16[sl].reshape(ntiles, T, S).transpose(0, 2, 1)
        ).reshape(ntiles * S, T)
        f_t = np.ascontiguousarray(
            f16[sl].reshape(ntiles, T, S, C).transpose(0, 2, 3, 1)
        ).reshape(ntiles * S, C * T)
        in_maps.append({"rho": rho_t, "f": f_t, "consts": cst})
    res = run_bass_kernel_spmd(nc, in_maps, list(range(NCORES)), trace=TRACE)
    LAST_RESULTS = res
    out = np.concatenate(
        [res.results[i]["out"] for i in range(NCORES)], axis=1
    )  # [C, N]
    return out.reshape(C, H, W)[None].astype(np.float32)


# revision 11
# speedup vs baseline: 1.0969x; 1.0969x over previous
"""Trainium2 Bass kernel for nn_DifferentiableProjector (volume rendering).

Math (per ray i, samples s=0..S-1, channels c):
    T_excl[s] = exp(-DT * sum_{s'<s} rho[s'])
    T_incl[s] = exp(-DT * sum_{s'<=s} rho[s'])
    w[s]      = T_excl[s] - T_incl[s]        (= T_excl * alpha)
    out[i,c]  = sum_s w[s] * f[i,s,c]

Sharding: data-parallel over rays, 65536 rays -> 8 cores x 8192 rays.

Design (all compute in "transposed space", s on partitions):
  - host casts rho/f to fp16 AND pre-tiles them so each 512-ray tile is a
    fully contiguous [S, C, T] DRAM slab (16 KB/partition rows) -> plain
    max-bandwidth DMAs, alternating the two HWDGE queues (sync/scalar)
  - cumsum over s (the partition axis) via triangular-ones matmuls on
    TensorE (fp32 PSUM); w = exp(-DT*excl) - exp(-DT*incl) with fp32 exps
    (fp16 exps would cancel catastrophically), cast to fp16
  - the big multiply is ONE DVE instruction per tile at 2x (fp16, w
    broadcast on the middle dim so the innermost stride stays 1);
    splitting it only pays the ~0.4us fixed DVE instruction cost 4x
  - segment-reduce over s: 16 accumulating one-hot matmuls on TensorE
    route channel-c column sums into PSUM row c -> [16, T] per tile;
    E_c weight loads are shared across tile pairs (also keeps the PE
    streaming dense, which matters for its DVFS p-state ramp)
  - ScalarE evacuates PSUM -> SBUF in fp16; outputs stream out per
    tile-pair (half the bytes of fp32; host upcasts after the gather)
  - rho preload is split so tile 0's cumsum isn't gated on the full
    2 MiB rho slab

v1 naive DVE: 415 us; v2 (4-way split mul): 116-129 us. This version:
DMA-bound at ~34 MiB/core over ~360 GB/s -> ~98 us floor + ramp/tail.
"""

import numpy as np

import concourse.bass as bass
import concourse.tile as tile
from concourse.bacc import Bacc
from concourse import mybir
from concourse.bass_utils import run_bass_kernel_spmd

H, W, S, C = 256, 256, 128, 16
N = H * W
NCORES = 8
NS = N // NCORES          # rays per core
P = 128                   # partitions (= S)
T = 512                   # rays per tile
DT = (6.0 - 2.0) / S

_cached = {}

# test-harness hooks (ignored by grading path)
TRACE = False
LAST_RESULTS = None

F16 = mybir.dt.float16
F32 = mybir.dt.float32


def _build_nc(ns: int = NS) -> bass.Bass:
    ntiles = ns // T
    nc = Bacc()
    # host supplies tensors pre-transposed AND pre-tiled:
    #   rho [ntiles*S, T]  (tile t rows t*S:(t+1)*S = rho[s, i] slab)
    #   f   [ntiles*S, C*T] (tile t rows = f[s, (c, i)] slab, contiguous)
    rho_d = nc.dram_tensor("rho", [ntiles * S, T], F16, kind="ExternalInput")
    f_d = nc.dram_tensor("f", [ntiles * S, C * T], F16, kind="ExternalInput")
    cst_d = nc.dram_tensor("consts", [P, 2 * P + C * C], F16, kind="ExternalInput")
    out_d = nc.dram_tensor("out", [C, ns], F16, kind="ExternalOutput")

    with tile.TileContext(nc) as tc:
        with (
            tc.tile_pool(name="cpool", bufs=1) as cpool,
            tc.tile_pool(name="fpool", bufs=6) as fpool,
            tc.tile_pool(name="tpool", bufs=3) as tpool,
            tc.tile_pool(name="spool", bufs=4) as spool,
            tc.tile_pool(name="opool", bufs=3) as opool,
            tc.tile_pool(name="psc", bufs=2, space="PSUM") as psc,
            tc.tile_pool(name="pso", bufs=2, space="PSUM") as pso,
        ):
            # consts/rho MUST go on the sync/scalar HWDGE rings: the GpSimd
            # ring turned out to run on a single DMA engine (~23 GB/s), so
            # a 2 MiB rho slab dispatched there takes ~90us and stalls
            # every tile's cumsum (v3-v6 regression, +26us)
            consts = cpool.tile([P, 2 * P + C * C], F16)
            nc.scalar.dma_start(out=consts, in_=cst_d[:, :])
            u_excl = consts[:, 0:P]
            u_incl = consts[:, P : 2 * P]
            # E_c = consts[:, 2P + 16c : 2P + 16c + 16]: column m one-hot at c
            e_base = 2 * P

            # rho slabs: [s, (tile, i)] fp16, split so the first pair of
            # tiles is ready ~8x sooner than the full slab
            rho_all = cpool.tile([P, ntiles, T], F16)
            rho_r = rho_d[:, :].rearrange("(t s) i -> s t i", s=S)
            nc.sync.dma_start(out=rho_all[:, 0:2, :], in_=rho_r[:, 0:2, :])
            nc.sync.dma_start(
                out=rho_all[:, 2:ntiles, :], in_=rho_r[:, 2:ntiles, :]
            )

            def tile_front(t, nsplit=1):
                """DMA load + w pipeline + big multiply for tile t. nsplit>1
                chunks the multiply so the reduce can start early (used for
                the last tiles, where the post-DMA tail is exposed).

                The f dispatch alternates sync/scalar rings and is issued
                inline (v2 pattern): engine instruction queues run ahead of
                compute, which keeps the HBM stream fed; explicit upfront
                pre-allocation + prefetch dispatch (v3-v5) made the tile
                framework serialize the dispatch streams and starved DMA."""
                fT = fpool.tile([P, C, T], F16, tag="fT")
                f_eng = nc.sync if t % 2 == 0 else nc.scalar
                f_slab = f_d[t * S : (t + 1) * S, :].rearrange(
                    "s (c i) -> s c i", c=C
                )
                f_eng.dma_start(out=fT, in_=f_slab)
                rhoT = rho_all[:, t, :]

                # cumsum over s (partition axis) via triangular matmuls
                pexc = psc.tile([P, T], F32, tag="pexc")
                pinc = psc.tile([P, T], F32, tag="pinc")
                nc.tensor.matmul(pexc, u_excl, rhoT, start=True, stop=True)
                nc.tensor.matmul(pinc, u_incl, rhoT, start=True, stop=True)

                # exps in fp32 (w = e1 - e2 cancels; fp16 here costs ~4% on w)
                e1 = spool.tile([P, T], F32, tag="e1")
                e2 = spool.tile([P, T], F32, tag="e2")
                nc.scalar.activation(
                    e1, pexc, mybir.ActivationFunctionType.Exp, scale=-DT
                )
                nc.scalar.activation(
                    e2, pinc, mybir.ActivationFunctionType.Exp, scale=-DT
                )
                # the sub stays on DVE: measured Pool tensor_tensor is ~4.3us
                # for this shape (4x the cost model), far too slow for the
                # w -> multiply critical path
                w = spool.tile([P, T], F16, tag="w")
                nc.vector.tensor_sub(w, e1, e2)

                # tmp[s, c, i] = fT[s, c, i] * w[s, i] on DVE at 2x
                tmp = tpool.tile([P, C, T], F16, tag="tmp")
                cs = C // nsplit
                for q in range(nsplit):
                    nc.vector.tensor_mul(
                        tmp[:, q * cs : (q + 1) * cs, :],
                        fT[:, q * cs : (q + 1) * cs, :],
                        w[:, None, :].broadcast_to((P, cs, T)),
                    )
                return tmp

            def tile_back(t, tmp_list):
                """Reduce + evacuate for 1-2 tiles, sharing each E_c weight
                load across the tiles' matmuls."""
                psums = [pso.tile([C, T], F32, tag=f"po{i}", name=f"po{i}_{t}")
                         for i in range(len(tmp_list))]
                for c in range(C):
                    lhs = consts[:, e_base + c * C : e_base + (c + 1) * C]
                    for tmp_t, po in zip(tmp_list, psums):
                        nc.tensor.matmul(
                            po,
                            lhs,
                            tmp_t[:, c, :],
                            start=(c == 0),
                            stop=(c == C - 1),
                        )
                nt = len(tmp_list)
                out_pair = opool.tile([C, nt * T], F16, tag="out_pair",
                                      name=f"out_pair_{t}")
                for k, po in enumerate(psums):
                    nc.scalar.activation(
                        out_pair[:, k * T : (k + 1) * T],
                        po,
                        mybir.ActivationFunctionType.Copy,
                    )
                # stream the output out now (overlaps later tiles); Scalar
                # dispatch right after its own evacs adds no new waits
                nc.scalar.dma_start(
                    out=out_d[:, t * T : (t + nt) * T],
                    in_=out_pair,
                )

            for t in range(0, ntiles - 2, 2):
                tmp_a = tile_front(t)
                tmp_b = tile_front(t + 1)
                tile_back(t, [tmp_a, tmp_b])
            # last two tiles: unpaired, chunked multiplies -> the reduce
            # interleaves with the multiply instead of trailing the last DMA
            tmp_a = tile_front(ntiles - 2, nsplit=4)
            tile_back(ntiles - 2, [tmp_a])
            tmp_b = tile_front(ntiles - 1, nsplit=4)
            tile_back(ntiles - 1, [tmp_b])
    if not nc.is_finalized():
        nc.finalize()
    return nc


def _consts() -> np.ndarray:
    u_excl = np.triu(np.ones((P, P), np.float16), 1)
    u_incl = np.triu(np.ones((P, P), np.float16), 0)
    # E[:, c*C + m] = 1 if m == c else 0  (all rows identical)
    e = np.tile(np.eye(C, dtype=np.float16).reshape(1, C * C), (P, 1))
    return np.ascontiguousarray(np.concatenate([u_excl, u_incl, e], axis=1))


def kernel(rho: np.ndarray, f: np.ndarray) -> np.ndarray:
    global LAST_RESULTS
    if "nc" not in _cached:
        _cached["nc"] = _build_nc()
        _cached["consts"] = _consts()
    nc = _cached["nc"]

    rho16 = np.asarray(rho, dtype=np.float16).reshape(N, S)
    f16 = np.asarray(f, dtype=np.float16).reshape(N, S, C)
    cst = _cached["consts"]
    ntiles = NS // T

    in_maps = []
    for i in range(NCORES):
        sl = slice(i * NS, (i + 1) * NS)
        # [rays, S] -> [ntiles, S, T] / [rays, S, C] -> [ntiles, S, C, T]
        rho_t = np.ascontiguousarray(
            rho16[sl].reshape(ntiles, T, S).transpose(0, 2, 1)
        ).reshape(ntiles * S, T)
        f_t = np.ascontiguousarray(
            f16[sl].reshape(ntiles, T, S, C).transpose(0, 2, 3, 1)
        ).reshape(ntiles * S, C * T)
        in_maps.append({"rho": rho_t, "f": f_t, "consts": cst})
    res = run_bass_kernel_spmd(nc, in_maps, list(range(NCORES)), trace=TRACE)
    LAST_RESULTS = res
    out = np.concatenate(
        [res.results[i]["out"] for i in range(NCORES)], axis=1
    )  # [C, N]
    return out.reshape(C, H, W)[None].astype(np.float32)
